# revision 41
# baseline (speedup 1.0000x reference)
"""AdaptiveSparseAttention on 8 TRN2 NeuronCores (Bass/Tile).

For the graded inputs the pattern-selector softmax yields pw ~ [0.34, 0.36,
0.30] per batch: pw[:,1] > THRESHOLD=0.1, so `combined > THRESHOLD` is true at
every (i, j). The binary mask is all-ones and the module reduces exactly to
dense softmax attention + output projection. The host verifies that condition
on the actual inputs (tiny MLP in numpy) and falls back to a full numpy
implementation of the reference semantics if it ever fails.

Sharding: core c <- (batch b = c//2, head group g = c%2 i.e. heads g*8..g*8+8).

Device schedule (per core):
- All DRAM tensors are host-packed partition-major so every DMA is an
  identity copy with multi-KB per-partition lines (full HBM bandwidth).
- A short scratch-fed warm-up matmul stream keeps the PE busy through the DMA
  ramp so the HAM clock gate reaches 8/8 before the real stream begins.
- Heads processed in pairs (2g, 2g+1) sharing e-tile g of qt/kt (head A on
  partitions 0-63, head B on 64-127). The pair's score matmuls are emitted
  back-to-back with disjoint PE row groups (K=64) and disjoint PSUM banks, so
  the 128x128 array runs them concurrently (~2x on scores).
- ch-major sub-steps (g, ch, jb): each sub-step computes one [128j x 512i]
  score block per head into its own single-bank PSUM tile (pool bufs=3, a
  3-deep pipeline), so the next score matmul never head-of-line blocks the
  in-order PE queue waiting on this sub-step's exp. The per-(head, ch)
  attn@V accumulator is a single-bank [65, 512] tile, which is what frees
  the banks for that 3-deep score pipeline (2 sm + 3 st + 3 o = 8 banks).
- attn@V trails scores/exp by a LAG-sub-step software pipeline; V carries an
  appended ones column so the softmax denominator falls out of the same
  accumulation (row 64 of the [65, 512] PSUM accumulator).
- Eviction is chunked per (pair, ch): denominator row copy (ScalarE,
  PSUM-close, [1, 512] so it interleaves between exps in the ACT queue),
  reciprocal, GpSimd partition-broadcast, then the normalize multiply reads
  o straight from PSUM (no staging copy) into osb.
- All independent work (v projection, later q/k e-tiles, output projection)
  drains as PE filler inside the attention sub-steps under cumulative
  earliest-deadline-first pacing, so the PE never idles long (idle >3.4us
  re-throttles the clock; sustained high draw can still drop the chip to
  the 2.0 GHz P0 state - that shows as a uniform ~20% run-to-run swing).
  Projection streams as pair groups evict: [0,1] -> yQ0 mid-kernel,
  [2] -> yQ2 with its deadline past the last step so leftovers keep the PE
  warm through the tail, and pair 3 -> yQ1 as two EDF fillers gated on the
  two pair-3 normalize halves (ch=0 overlaps the last sub-steps, ch=1 the
  flush). Tail accumulators round-robin across all three PSUM pools so the
  16 tail matmuls never wait on an eviction cast, and all late stores issue
  on HWDGE (sync) queues - SWDGE stores at the tail stretch the final drain.
- Host sums the three bf16 partials, transposes, adds b_proj.
"""

import math

import numpy as np

B, L, D = 4, 1024, 1024
H = 16
HD = D // H
SCALE = HD ** -0.5
LOCAL_WINDOW = 32
HALF = LOCAL_WINDOW // 2
THRESHOLD = 0.1
SPARSITY_RATIO = 0.3
EFF = min(SPARSITY_RATIO, 1.0 - 10.0 / L)
K_TOP = max(1, min(L, int(L * (1.0 - EFF))))

N_CORES = 8
P = 128
NB = L // P  # 8 blocks of 128 along L
HPC = H // 2  # heads per core (8)
DPC = HPC * HD  # d-range per core (512)
LAG = 2  # attn@V trails scores/exp by this many sub-steps
WQW = 3 * DPC  # packed qkv weight row width (per k-block)
VOFF = 4 * 2 * P  # column offset of the v section in the packed qkv weights

_CACHE = {}


# ----------------------------------------------------------------------------
# host-side reference pieces (pattern selector + full fallback)
# ----------------------------------------------------------------------------

_erf = np.vectorize(math.erf)


def _gelu(x):
    return x * 0.5 * (1.0 + _erf(x / np.sqrt(2.0)))


def _pattern_weights(x, ps_w1, ps_b1, ln_g, ln_b, ps_w2, ps_b2, ps_w3, ps_b3,
                     pattern_bias):
    x = np.asarray(x, np.float64)
    pooled = (np.mean(x, axis=1) + np.max(x, axis=1)) / 2.0
    h = pooled @ np.asarray(ps_w1, np.float64).T + ps_b1
    mu = np.mean(h, axis=-1, keepdims=True)
    var = np.mean((h - mu) ** 2, axis=-1, keepdims=True)
    h = (h - mu) / np.sqrt(var + 1e-5) * ln_g + ln_b
    h = _gelu(h)
    h = _gelu(h @ np.asarray(ps_w2, np.float64).T + ps_b2)
    logits = h @ np.asarray(ps_w3, np.float64).T + ps_b3 + pattern_bias
    e = np.exp(logits - logits.max(-1, keepdims=True))
    return e / e.sum(-1, keepdims=True)  # [B, 3]


def _numpy_reference(x, w_qkv, w_proj, b_proj, ps_w1, ps_b1, ln_g, ln_b,
                     ps_w2, ps_b2, ps_w3, ps_b3, pattern_bias, sparse_w,
                     sparse_b):
    """Full reference semantics in numpy (slow fallback, general masks)."""
    x = np.asarray(x, np.float32)
    qkv = (x @ np.asarray(w_qkv, np.float32).T).reshape(B, L, 3, H, HD)
    qkv = np.transpose(qkv, (2, 0, 3, 1, 4))
    q, k, v = qkv[0], qkv[1], qkv[2]
    scores = np.einsum('bhid,bhjd->bhij', q, k).astype(np.float32) * SCALE

    pw = _pattern_weights(x, ps_w1, ps_b1, ln_g, ln_b, ps_w2, ps_b2, ps_w3,
                          ps_b3, pattern_bias).astype(np.float32)

    s2 = scores * np.abs(np.asarray(sparse_w, np.float32))[None] + \
        np.asarray(sparse_b, np.float32)[None]
    part = np.argpartition(-s2, K_TOP - 1, axis=-1)[..., :K_TOP]
    sparse_mask = np.zeros_like(scores)
    np.put_along_axis(sparse_mask, part, 1.0, axis=-1)

    i = np.arange(L)
    local = (np.abs(i[:, None] - i[None, :]) <= HALF).astype(np.float32)
    combined = (pw[:, 0, None, None, None] * local[None, None]
                + pw[:, 1, None, None, None]
                + pw[:, 2, None, None, None] * sparse_mask)
    binary = combined > THRESHOLD
    scores = np.where(binary, scores, -np.inf)
    all_masked = ~np.any(binary, axis=-1)
    eye = np.eye(L, dtype=bool)
    scores = np.where(all_masked[..., None] & eye[None, None], 0.0, scores)

    m = scores.max(-1, keepdims=True)
    ex = np.exp(scores - m)
    attn = ex / ex.sum(-1, keepdims=True)
    out = np.einsum('bhij,bhjd->bhid', attn, v)
    out = np.transpose(out, (0, 2, 1, 3)).reshape(B, L, D)
    return (out @ np.asarray(w_proj, np.float32).T + b_proj).astype(np.float32)


def _pmajor(a, p=P):
    """[R, C] -> [p, (R//p)*C]: row-block r lands at columns r*C:(r+1)*C."""
    r, c = a.shape
    return np.ascontiguousarray(
        a.reshape(r // p, p, c).transpose(1, 0, 2).reshape(p, (r // p) * c))


def _unpmajor(a, rows):
    pp, w = a.shape
    c = w // (rows // pp)
    return a.reshape(pp, rows // pp, c).transpose(1, 0, 2).reshape(rows, c)


# ----------------------------------------------------------------------------
# device kernel
# ----------------------------------------------------------------------------

def _build_nc():
    import concourse.bacc as bacc
    import concourse.mybir as mybir
    import concourse.tile as tile

    BF = mybir.dt.bfloat16
    F32 = mybir.dt.float32
    Exp = mybir.ActivationFunctionType.Exp

    nc = bacc.Bacc("TRN2", target_bir_lowering=False, debug=False,
                   num_devices=N_CORES)

    # all tensors partition-major: [128, blocks * width]
    xt_d = nc.dram_tensor("xtP", [P, NB * L], BF, kind="ExternalInput")
    wqkv_d = nc.dram_tensor("wqkvP", [P, NB * WQW], BF, kind="ExternalInput")
    wproj_d = nc.dram_tensor("wprojP", [P, 4 * D], BF, kind="ExternalInput")
    yq_d = [nc.dram_tensor(f"yQ{i}", [P, NB * L], BF, kind="ExternalOutput")
            for i in range(3)]

    with tile.TileContext(nc) as tc:
        with (
            tc.tile_pool(name="pers", bufs=1) as pers,
            tc.tile_pool(name="work", bufs=2) as work,
            tc.tile_pool(name="pt", bufs=16) as ptp,
            tc.tile_pool(name="ys", bufs=6) as ysp,
            tc.tile_pool(name="ps_sm", bufs=2, space="PSUM") as ps_sm,
            tc.tile_pool(name="ps_st", bufs=3, space="PSUM") as ps_st,
            tc.tile_pool(name="ps_o", bufs=3, space="PSUM") as ps_o,
        ):
            # ---- persistent SBUF tensors ----
            xtw = pers.tile([P, NB * L], BF, name="xtw")
            wqw = pers.tile([P, NB * WQW], BF, name="wqw")
            wpw = pers.tile([P, 4 * D], BF, name="wpw")
            scr = pers.tile([P, 512], BF, name="scratch")
            qt = [pers.tile([P, L], BF, name=f"qt{i}", tag=f"qt{i}")
                  for i in range(4)]
            kt = [pers.tile([P, L], BF, name=f"kt{i}", tag=f"kt{i}")
                  for i in range(4)]
            VW = HD + 1  # [v (64) | ones] per head
            vsb = [pers.tile([P, HPC, VW], BF, name=f"v{i}", tag=f"v{i}")
                   for i in range(NB)]
            osb = [pers.tile([P, L], BF, name=f"o{i}", tag=f"o{i}")
                   for i in range(4)]

            def xts(kb, lo, size):
                return xtw[:, kb * L + lo:kb * L + lo + size]

            def wqs(kb, lo, size):
                return wqw[:, kb * WQW + lo:kb * WQW + lo + size]

            def wps(db, lo, size):
                return wpw[:, db * D + lo:db * D + lo + size]

            # ---- PE warm-up: scratch-fed matmuls through the DMA ramp so
            # the HAM clock gate reaches 8/8 before the real stream ----
            nc.vector.memset(scr[:], 0.0)

            def junk(n):
                for _ in range(n):
                    t = ps_st.tile([P, 512], F32, name="st", tag="st")
                    nc.tensor.matmul(t[:], scr[:, 0:P], scr[:],
                                     start=True, stop=True)

            junk(10)

            # ---- input DMAs (identity copies, issued sync/gpsimd) ----
            # per k-block first: x then the qk-e-tile-0 slice, so the first
            # QKV accumulation ramps with the DMA stream; v weights next
            # (attn@V needs them by ~step LAG), then the rest.
            engs = [nc.sync, nc.gpsimd]
            q = 0

            def dma(dst, src):
                nonlocal q
                engs[q % 2].dma_start(dst, src)
                q += 1

            for kb in range(NB):
                dma(xtw[:, kb * L:(kb + 1) * L],
                    xt_d[:, kb * L:(kb + 1) * L])
                dma(wqs(kb, 0, 2 * P),
                    wqkv_d[:, kb * WQW:kb * WQW + 2 * P])
            for kb in range(NB):
                dma(wqs(kb, VOFF, DPC),
                    wqkv_d[:, kb * WQW + VOFF:kb * WQW + VOFF + DPC])
            for kb in range(NB):
                dma(wqs(kb, 2 * P, VOFF - 2 * P),
                    wqkv_d[:, kb * WQW + 2 * P:kb * WQW + VOFF])
            dma(wpw[:], wproj_d[:])

            # ---- generators for fill work ----
            def emit_qk_et(et):
                for which, dst in ((0, qt), (1, kt)):
                    for ch in range(2):
                        acc = ps_sm.tile([P, 512], F32, name="psqk", tag="ps")

                        def mk(acc=acc, which=which, et=et, ch=ch):
                            for kb in range(NB):
                                yield lambda kb=kb, acc=acc, which=which, \
                                    et=et, ch=ch: nc.tensor.matmul(
                                    acc[:],
                                    wqs(kb, et * 2 * P + which * P, P),
                                    xts(kb, ch * 512, 512),
                                    start=(kb == 0), stop=(kb == NB - 1),
                                )
                        yield from mk()
                        if et == 0:
                            yield lambda dst=dst, et=et, ch=ch, acc=acc: \
                                nc.scalar.copy(
                                    dst[et][:, ch * 512:(ch + 1) * 512],
                                    acc[:])
                        else:
                            yield lambda dst=dst, et=et, ch=ch, acc=acc: \
                                nc.vector.tensor_copy(
                                    dst[et][:, ch * 512:(ch + 1) * 512],
                                    acc[:])

            def emit_v(lb):
                yield lambda lb=lb: nc.vector.memset(
                    vsb[lb][:, :, HD:VW], 1.0)
                acc = ps_sm.tile([P, DPC], F32, name="psv", tag="ps")

                def mk(acc=acc, lb=lb):
                    for kb in range(NB):
                        yield lambda kb=kb, acc=acc, lb=lb: nc.tensor.matmul(
                            acc[:],
                            xts(kb, lb * P, P),
                            wqs(kb, VOFF, DPC),
                            start=(kb == 0), stop=(kb == NB - 1),
                        )
                yield from mk()
                yield lambda acc=acc, lb=lb: nc.vector.tensor_copy(
                    vsb[lb][:, :, 0:HD],
                    acc[:].rearrange("p (h d) -> p h d", h=HPC),
                )

            # Output blocks are packed ch-outer (block = ch*NB + et, host
            # re-orders), so consecutive et tiles of a quarter are adjacent
            # in DRAM: two casts stage into one [128, 1024] buffer and ship
            # as a single store - half the DMA issues.
            def emit_proj(dbs, ydst, sync_store):
                # ch=0 tiles first: the ch=1 normalize multiplies of the
                # gating pair have just been emitted when this generator
                # opens, so ch=1 readers must trail them by several tiles.
                stage = [None]
                for u, (ch, et) in enumerate(
                        (ch, et) for ch in range(2) for et in range(NB)):
                    acc = ps_sm.tile([P, 512], F32, name="psy", tag="ps")
                    for db in dbs:
                        yield lambda acc=acc, db=db, et=et, ch=ch, dbs=dbs: \
                            nc.tensor.matmul(
                                acc[:],
                                wps(db, et * P, P),
                                osb[db][:, ch * 512:(ch + 1) * 512],
                                start=(db == dbs[0]), stop=(db == dbs[-1]),
                            )

                    def fin(acc=acc, et=et, ch=ch, ydst=ydst, u=u,
                            sync_store=sync_store, stage=stage):
                        if et % 2 == 0:
                            stage[0] = ysp.tile([P, 1024], BF, name="ystage",
                                                tag="ystage")
                        dst = stage[0][:, (et % 2) * 512:(et % 2 + 1) * 512]
                        # the late quarter's ch=1 tiles drain in the flush,
                        # when ScalarE is done with exps - offload its casts
                        # there to unclog the DVE cast pipe in that window
                        if sync_store and ch == 1:
                            nc.scalar.copy(dst, acc[:])
                        else:
                            nc.vector.tensor_copy(dst, acc[:])
                        if et % 2 == 1:
                            eng = (nc.sync if (sync_store or (u // 2) % 2)
                                   else nc.gpsimd)
                            eng.dma_start(
                                ydst[:, (ch * NB + et - 1) * 512:
                                     (ch * NB + et + 1) * 512],
                                stage[0][:])
                    yield fin

            # pair-3 projection accumulators round-robin across all three
            # PSUM pools (score and oacc slots are dead by the tail) so the
            # matmuls never wait on an eviction cast.
            tail_pools = [ps_sm, ps_st, ps_o]
            tail_tags = ["ps", "st", "ot"]

            def emit_proj3g(ch):
                # ch=0 runs while scores still cycle the st pool and pair-3
                # ch=1 still accumulates in the ot pool - keep its
                # accumulators in the sm pool. ch=1 runs in the flush when
                # both other pools are dead.
                stage = [None]
                for et in range(NB):
                    pi = 0 if ch == 0 else 1 + et % 2
                    acc = tail_pools[pi].tile([P, 512], F32, name="psy3",
                                              tag=tail_tags[pi])
                    yield lambda acc=acc, et=et, ch=ch: nc.tensor.matmul(
                        acc[:], wps(3, et * P, P),
                        osb[3][:, ch * 512:(ch + 1) * 512],
                        start=True, stop=True,
                    )

                    def fin(acc=acc, et=et, ch=ch, stage=stage):
                        if et % 2 == 0:
                            stage[0] = ysp.tile([P, 1024], BF, name="ystage",
                                                tag="ystage")
                        if ch == 1 and et % 2:
                            nc.scalar.copy(
                                stage[0][:, 512:1024], acc[:])
                        else:
                            nc.vector.tensor_copy(
                                stage[0][:, (et % 2) * 512:
                                         (et % 2 + 1) * 512],
                                acc[:])
                        if et % 2 == 1:
                            nc.sync.dma_start(
                                yq_d[1][:, (ch * NB + et - 1) * 512:
                                        (ch * NB + et + 1) * 512],
                                stage[0][:])
                    yield fin

            # ---- EDF filler scheduling ----
            # fillers: [deadline_substep, ops_left, gate, gen]; gate 0-3
            # compares against pairs evicted, gate 4/5 against pair-3
            # normalize halves done.
            NHS = 64
            evicted = [0]
            p3 = [0]
            fillers = []
            for jb in range(NB):
                fillers.append([max(1, LAG + jb - 1), 10, 0, emit_v(jb)])
            for g in (1, 2, 3):
                fillers.append([16 * g - 2, 36, 0, emit_qk_et(g)])
            # proj quarters stream as their pair groups evict; quarter [2]'s
            # deadline sits past the last step so leftovers keep the PE warm
            # through the tail eviction.
            fillers.append([54, 48, 2, emit_proj([0, 1], yq_d[0], False)])
            fillers.append([NHS + 10, 32, 3, emit_proj([2], yq_d[2], True)])
            fillers.append([NHS + 4, 16, 4, emit_proj3g(0)])
            fillers.append([NHS + 12, 16, 5, emit_proj3g(1)])

            def gate_ok(f):
                g = f[2]
                return evicted[0] >= g if g <= 3 else p3[0] >= g - 3

            def drain_one():
                for f in fillers:
                    if f[1] == 0 or not gate_ok(f):
                        continue
                    fn = next(f[3], None)
                    if fn is None:
                        f[1] = 0
                        continue
                    fn()
                    f[1] -= 1
                    return True
                return False

            def drain_for(t):
                for f in fillers:
                    while f[0] <= t and f[1] > 0 and gate_ok(f):
                        fn = next(f[3], None)
                        if fn is None:
                            f[1] = 0
                            break
                        fn()
                        f[1] -= 1
                due = sorted((f for f in fillers
                              if f[1] > 0 and gate_ok(f)),
                             key=lambda f: f[0])
                n = 2
                cum = 0
                for f in due:
                    cum += f[1]
                    if f[0] > t:
                        n = max(n, -(-cum // (f[0] - t)))
                for _ in range(min(n, 9)):
                    if not drain_one():
                        break

            # ---- phase 1: q/k e-tile 0 ----
            # kb-outer with all four accumulators live (2 sm + 2 st slots):
            # each k-block's four matmuls consume its x/weight DMAs right as
            # they land instead of re-traversing all blocks per output, and
            # each weight load serves both i-chunks back-to-back.
            p1acc = {}
            for which in range(2):
                for ch in range(2):
                    pool, tag = ((ps_sm, "ps") if which == 0
                                 else (ps_st, "st"))
                    p1acc[(which, ch)] = pool.tile([P, 512], F32,
                                                   name="psqk0", tag=tag)
            for kb in range(NB):
                for which in range(2):
                    for ch in range(2):
                        nc.tensor.matmul(
                            p1acc[(which, ch)][:],
                            wqs(kb, which * P, P),
                            xts(kb, ch * 512, 512),
                            start=(kb == 0), stop=(kb == NB - 1),
                        )
            for which, dst in ((0, qt), (1, kt)):
                for ch in range(2):
                    nc.scalar.copy(dst[0][:, ch * 512:(ch + 1) * 512],
                                   p1acc[(which, ch)][:])

            # ---- phase 2: paired attention with lagged attn@V ----
            # ch-major sub-steps: each (g, ch, jb) sub-step computes one
            # [128j x 512i] score block for both heads. The per-(head, ch)
            # attn@V accumulator is a single-bank [65, 512] PSUM tile, which
            # frees banks for a 3-deep score-tile pipeline (the next score
            # matmul never waits on this sub-step's exp).
            substeps = [(g, ch, jb)
                        for g in range(4) for ch in range(2)
                        for jb in range(NB)]
            emis = {}
            for idx, (g, ch, jb) in enumerate(substeps):
                at = idx + LAG + (1 if jb == 0 else 0)
                emis.setdefault(at, []).append((g, ch, jb))
            ptt = {}
            oacc = {}

            def emit_attnv(g2, ch2, jb2):
                for h, hi in ((2 * g2, 0), (2 * g2 + 1, 1)):
                    if jb2 == 0:
                        oacc[(h, ch2)] = ps_o.tile([VW, 512], F32,
                                                   name="ot", tag="ot")
                    pt = ptt.pop((g2, ch2, jb2, hi))
                    nc.tensor.matmul(
                        oacc[(h, ch2)][:],
                        vsb[jb2][:, h, :],
                        pt[:],
                        start=(jb2 == 0), stop=(jb2 == NB - 1),
                    )

            def emit_evict_chunk(g2, ch2):
                # Normalize the ch-half of both heads of pair g2 straight out
                # of their PSUM accumulators (no staging copy).
                sl = slice(ch2 * 512, (ch2 + 1) * 512)
                for h in (2 * g2, 2 * g2 + 1):
                    base = (h % 2) * HD
                    acc = oacc[(h, ch2)]
                    sums = work.tile([1, 512], F32, name="sums", tag="sums")
                    # DVE is the congested engine in the tail pair's window
                    # (muls + projection casts); ScalarE's exp stream is
                    # nearly drained there, so its PSUM-close copy is free
                    if g2 == 3:
                        nc.scalar.copy(sums[:], acc[HD:HD + 1, :])
                    else:
                        nc.vector.tensor_copy(sums[:], acc[HD:HD + 1, :])
                    inv = work.tile([1, 512], F32, name="inv", tag="inv")
                    nc.vector.reciprocal_approx_fast(inv[:], sums[:])
                    invbc = work.tile([HD, 512], F32, name="invbc",
                                      tag="invbc")
                    nc.gpsimd.partition_broadcast(invbc[:], inv[:])
                    nc.vector.tensor_mul(
                        osb[g2][base:base + HD, sl],
                        acc[0:HD, :],
                        invbc[:],
                    )
                if ch2 == 1:
                    evicted[0] = g2 + 1
                if g2 == 3:
                    p3[0] = ch2 + 1

            for si, (g, ch, jb) in enumerate(substeps):
                tq, tk = qt[g], kt[g]
                for g2, ch2, jb2 in emis.get(si, ()):
                    emit_attnv(g2, ch2, jb2)
                    if jb2 == NB - 1:
                        emit_evict_chunk(g2, ch2)
                for hi, rows in ((0, slice(0, HD)), (1, slice(HD, P))):
                    stx = ps_st.tile([P, 512], F32, name="st", tag="st")
                    nc.tensor.matmul(
                        stx[:],
                        tk[rows, jb * P:(jb + 1) * P],
                        tq[rows, ch * 512:(ch + 1) * 512],
                        start=True, stop=True,
                    )
                    pt = ptp.tile([P, 512], BF, name="pt", tag="pt")
                    nc.scalar.activation(pt[:], stx[:], Exp)
                    ptt[(g, ch, jb, hi)] = pt
                drain_for(si)

            # ---- phase 3: flush lag, final evict + tail projection ----
            for si in range(len(substeps), len(substeps) + LAG + 2):
                for g2, ch2, jb2 in emis.get(si, ()):
                    emit_attnv(g2, ch2, jb2)
                    if jb2 == NB - 1:
                        emit_evict_chunk(g2, ch2)
                    for _ in range(3):
                        drain_one()
            while any(f[1] for f in fillers if gate_ok(f)):
                if not drain_one():
                    break

    nc.compile()
    return nc


def _get_nc():
    if "nc" not in _CACHE:
        _CACHE["nc"] = _build_nc()
    return _CACHE["nc"]


def kernel(x, w_qkv, w_proj, b_proj, ps_w1, ps_b1, ln_g, ln_b, ps_w2, ps_b2,
           ps_w3, ps_b3, pattern_bias, sparse_w, sparse_b):
    import concourse.mybir as mybir
    from concourse.bass_utils import run_bass_kernel_spmd

    pw = _pattern_weights(x, ps_w1, ps_b1, ln_g, ln_b, ps_w2, ps_b2, ps_w3,
                          ps_b3, pattern_bias)
    if pw[:, 1].min() <= THRESHOLD + 1e-4:
        # mask not provably dense -> exact (slow) fallback
        return _numpy_reference(x, w_qkv, w_proj, b_proj, ps_w1, ps_b1, ln_g,
                                ln_b, ps_w2, ps_b2, ps_w3, ps_b3, pattern_bias,
                                sparse_w, sparse_b)

    bf16 = mybir.dt.np(mybir.dt.bfloat16)
    x = np.asarray(x, np.float32)
    w_qkv = np.asarray(w_qkv, np.float32)
    w_proj = np.asarray(w_proj, np.float32)

    in_maps = []
    for c in range(N_CORES):
        b = c // 2
        h0 = (c % 2) * HPC
        rq = slice(h0 * HD, (h0 + HPC) * HD)
        wqc = w_qkv[0 * D:1 * D][rq] * SCALE           # [512, 1024]
        wkc = w_qkv[1 * D:2 * D][rq]
        wvc = w_qkv[2 * D:3 * D][rq]
        # packed row order: [q_et0, k_et0, q_et1, k_et1, ..., v]
        secs = []
        for et in range(4):
            secs.append(wqc[et * P:(et + 1) * P])
            secs.append(wkc[et * P:(et + 1) * P])
        secs.append(wvc)
        wqkvT = np.concatenate(secs, 0).T.astype(bf16)          # [1024, 1536]
        wprojT = w_proj.T[rq, :].astype(bf16)                   # [512, 1024]
        xt = x[b].T.astype(bf16)                                # [1024, 1024]
        in_maps.append({"xtP": _pmajor(xt), "wqkvP": _pmajor(wqkvT),
                        "wprojP": _pmajor(wprojT)})

    res = run_bass_kernel_spmd(_get_nc(), in_maps, list(range(N_CORES)),
                               trace=bool(_CACHE.get("trace", False)))
    _CACHE["last_exec_time_ns"] = res.exec_time_ns
    _CACHE["last_res"] = res

    out = np.empty((B, L, D), np.float32)
    bp = np.asarray(b_proj, np.float32)
    for b in range(B):
        yt = np.zeros((P, NB * L), np.float32)
        for c in (2 * b, 2 * b + 1):
            r = res.results[c]
            for i in range(3):
                yt += r[f"yQ{i}"].astype(np.float32)
        # device packs output blocks ch-outer: block = ch*NB + et
        yt = yt.reshape(P, 2, NB, 512).transpose(0, 2, 1, 3).reshape(
            P, NB * L)
        out[b] = _unpmajor(yt, D).T + bp[None, :]
    return out


# revision 42
# speedup vs baseline: 1.0265x; 1.0265x over previous
"""AdaptiveSparseAttention on 8 TRN2 NeuronCores (Bass/Tile).

For the graded inputs the pattern-selector softmax yields pw ~ [0.34, 0.36,
0.30] per batch: pw[:,1] > THRESHOLD=0.1, so `combined > THRESHOLD` is true at
every (i, j). The binary mask is all-ones and the module reduces exactly to
dense softmax attention + output projection. The host verifies that condition
on the actual inputs (tiny MLP in numpy) and falls back to a full numpy
implementation of the reference semantics if it ever fails.

Sharding: core c <- (batch b = c//2, head group g = c%2 i.e. heads g*8..g*8+8).

Device schedule (per core):
- All DRAM tensors are host-packed partition-major so every DMA is an
  identity copy with multi-KB per-partition lines (full HBM bandwidth).
- A short scratch-fed warm-up matmul stream keeps the PE busy through the DMA
  ramp so the HAM clock gate reaches 8/8 before the real stream begins.
- Heads processed in pairs (2g, 2g+1) sharing e-tile g of qt/kt (head A on
  partitions 0-63, head B on 64-127). The pair's score matmuls are emitted
  back-to-back with disjoint PE row groups (K=64) and disjoint PSUM banks, so
  the 128x128 array runs them concurrently (~2x on scores).
- ch-major sub-steps (g, ch, jb): each sub-step computes one [128j x 512i]
  score block per head into its own single-bank PSUM tile (pool bufs=3, a
  3-deep pipeline), so the next score matmul never head-of-line blocks the
  in-order PE queue waiting on this sub-step's exp. The per-(head, ch)
  attn@V accumulator is a single-bank [65, 512] tile, which is what frees
  the banks for that 3-deep score pipeline (2 sm + 3 st + 3 o = 8 banks).
- attn@V trails scores/exp by a LAG-sub-step software pipeline; V carries an
  appended ones column so the softmax denominator falls out of the same
  accumulation (row 64 of the [65, 512] PSUM accumulator).
- Eviction is chunked per (pair, ch): denominator row copy (ScalarE,
  PSUM-close, [1, 512] so it interleaves between exps in the ACT queue),
  reciprocal, GpSimd partition-broadcast, then the normalize multiply reads
  o straight from PSUM (no staging copy) into osb.
- All independent work (v projection, later q/k e-tiles, output projection)
  drains as PE filler inside the attention sub-steps under cumulative
  earliest-deadline-first pacing, so the PE never idles long (idle >3.4us
  re-throttles the clock; sustained high draw can still drop the chip to
  the 2.0 GHz P0 state - that shows as a uniform ~20% run-to-run swing).
  Projection streams as pair groups evict: [0,1] -> yQ0 mid-kernel,
  [2] -> yQ2 with its deadline past the last step so leftovers keep the PE
  warm through the tail, and pair 3 -> yQ1 as two EDF fillers gated on the
  two pair-3 normalize halves (ch=0 overlaps the last sub-steps, ch=1 the
  flush). Tail accumulators round-robin across all three PSUM pools so the
  16 tail matmuls never wait on an eviction cast, and all late stores issue
  on HWDGE (sync) queues - SWDGE stores at the tail stretch the final drain.
- Host sums the three bf16 partials, transposes, adds b_proj.
"""

import math

import numpy as np

B, L, D = 4, 1024, 1024
H = 16
HD = D // H
SCALE = HD ** -0.5
LOCAL_WINDOW = 32
HALF = LOCAL_WINDOW // 2
THRESHOLD = 0.1
SPARSITY_RATIO = 0.3
EFF = min(SPARSITY_RATIO, 1.0 - 10.0 / L)
K_TOP = max(1, min(L, int(L * (1.0 - EFF))))

N_CORES = 8
P = 128
NB = L // P  # 8 blocks of 128 along L
HPC = H // 2  # heads per core (8)
DPC = HPC * HD  # d-range per core (512)
LAG = 2  # attn@V trails scores/exp by this many sub-steps
WQW = 3 * DPC  # packed qkv weight row width (per k-block)
VOFF = 4 * 2 * P  # column offset of the v section in the packed qkv weights

_CACHE = {}


# ----------------------------------------------------------------------------
# host-side reference pieces (pattern selector + full fallback)
# ----------------------------------------------------------------------------

_erf = np.vectorize(math.erf)


def _gelu(x):
    return x * 0.5 * (1.0 + _erf(x / np.sqrt(2.0)))


def _pattern_weights(x, ps_w1, ps_b1, ln_g, ln_b, ps_w2, ps_b2, ps_w3, ps_b3,
                     pattern_bias):
    x = np.asarray(x, np.float64)
    pooled = (np.mean(x, axis=1) + np.max(x, axis=1)) / 2.0
    h = pooled @ np.asarray(ps_w1, np.float64).T + ps_b1
    mu = np.mean(h, axis=-1, keepdims=True)
    var = np.mean((h - mu) ** 2, axis=-1, keepdims=True)
    h = (h - mu) / np.sqrt(var + 1e-5) * ln_g + ln_b
    h = _gelu(h)
    h = _gelu(h @ np.asarray(ps_w2, np.float64).T + ps_b2)
    logits = h @ np.asarray(ps_w3, np.float64).T + ps_b3 + pattern_bias
    e = np.exp(logits - logits.max(-1, keepdims=True))
    return e / e.sum(-1, keepdims=True)  # [B, 3]


def _numpy_reference(x, w_qkv, w_proj, b_proj, ps_w1, ps_b1, ln_g, ln_b,
                     ps_w2, ps_b2, ps_w3, ps_b3, pattern_bias, sparse_w,
                     sparse_b):
    """Full reference semantics in numpy (slow fallback, general masks)."""
    x = np.asarray(x, np.float32)
    qkv = (x @ np.asarray(w_qkv, np.float32).T).reshape(B, L, 3, H, HD)
    qkv = np.transpose(qkv, (2, 0, 3, 1, 4))
    q, k, v = qkv[0], qkv[1], qkv[2]
    scores = np.einsum('bhid,bhjd->bhij', q, k).astype(np.float32) * SCALE

    pw = _pattern_weights(x, ps_w1, ps_b1, ln_g, ln_b, ps_w2, ps_b2, ps_w3,
                          ps_b3, pattern_bias).astype(np.float32)

    s2 = scores * np.abs(np.asarray(sparse_w, np.float32))[None] + \
        np.asarray(sparse_b, np.float32)[None]
    part = np.argpartition(-s2, K_TOP - 1, axis=-1)[..., :K_TOP]
    sparse_mask = np.zeros_like(scores)
    np.put_along_axis(sparse_mask, part, 1.0, axis=-1)

    i = np.arange(L)
    local = (np.abs(i[:, None] - i[None, :]) <= HALF).astype(np.float32)
    combined = (pw[:, 0, None, None, None] * local[None, None]
                + pw[:, 1, None, None, None]
                + pw[:, 2, None, None, None] * sparse_mask)
    binary = combined > THRESHOLD
    scores = np.where(binary, scores, -np.inf)
    all_masked = ~np.any(binary, axis=-1)
    eye = np.eye(L, dtype=bool)
    scores = np.where(all_masked[..., None] & eye[None, None], 0.0, scores)

    m = scores.max(-1, keepdims=True)
    ex = np.exp(scores - m)
    attn = ex / ex.sum(-1, keepdims=True)
    out = np.einsum('bhij,bhjd->bhid', attn, v)
    out = np.transpose(out, (0, 2, 1, 3)).reshape(B, L, D)
    return (out @ np.asarray(w_proj, np.float32).T + b_proj).astype(np.float32)


def _pmajor(a, p=P):
    """[R, C] -> [p, (R//p)*C]: row-block r lands at columns r*C:(r+1)*C."""
    r, c = a.shape
    return np.ascontiguousarray(
        a.reshape(r // p, p, c).transpose(1, 0, 2).reshape(p, (r // p) * c))


def _unpmajor(a, rows):
    pp, w = a.shape
    c = w // (rows // pp)
    return a.reshape(pp, rows // pp, c).transpose(1, 0, 2).reshape(rows, c)


# ----------------------------------------------------------------------------
# device kernel
# ----------------------------------------------------------------------------

def _build_nc():
    import concourse.bacc as bacc
    import concourse.mybir as mybir
    import concourse.tile as tile

    BF = mybir.dt.bfloat16
    F32 = mybir.dt.float32
    Exp = mybir.ActivationFunctionType.Exp

    nc = bacc.Bacc("TRN2", target_bir_lowering=False, debug=False,
                   num_devices=N_CORES)

    # all tensors partition-major: [128, blocks * width]
    xt_d = nc.dram_tensor("xtP", [P, NB * L], BF, kind="ExternalInput")
    wqkv_d = nc.dram_tensor("wqkvP", [P, NB * WQW], BF, kind="ExternalInput")
    wproj_d = nc.dram_tensor("wprojP", [P, 4 * D], BF, kind="ExternalInput")
    yq_d = [nc.dram_tensor(f"yQ{i}", [P, NB * L], BF, kind="ExternalOutput")
            for i in range(3)]

    with tile.TileContext(nc) as tc:
        with (
            tc.tile_pool(name="pers", bufs=1) as pers,
            tc.tile_pool(name="work", bufs=2) as work,
            tc.tile_pool(name="pt", bufs=16) as ptp,
            tc.tile_pool(name="ys", bufs=6) as ysp,
            tc.tile_pool(name="ps_sm", bufs=2, space="PSUM") as ps_sm,
            tc.tile_pool(name="ps_st", bufs=3, space="PSUM") as ps_st,
            tc.tile_pool(name="ps_o", bufs=3, space="PSUM") as ps_o,
        ):
            # ---- persistent SBUF tensors ----
            xtw = pers.tile([P, NB * L], BF, name="xtw")
            wqw = pers.tile([P, NB * WQW], BF, name="wqw")
            wpw = pers.tile([P, 4 * D], BF, name="wpw")
            scr = pers.tile([P, 512], BF, name="scratch")
            qt = [pers.tile([P, L], BF, name=f"qt{i}", tag=f"qt{i}")
                  for i in range(4)]
            kt = [pers.tile([P, L], BF, name=f"kt{i}", tag=f"kt{i}")
                  for i in range(4)]
            VW = HD + 1  # [v (64) | ones] per head
            vsb = [pers.tile([P, HPC, VW], BF, name=f"v{i}", tag=f"v{i}")
                   for i in range(NB)]
            osb = [pers.tile([P, L], BF, name=f"o{i}", tag=f"o{i}")
                   for i in range(4)]

            def xts(kb, lo, size):
                return xtw[:, kb * L + lo:kb * L + lo + size]

            def wqs(kb, lo, size):
                return wqw[:, kb * WQW + lo:kb * WQW + lo + size]

            def wps(db, lo, size):
                return wpw[:, db * D + lo:db * D + lo + size]

            # ---- PE warm-up: scratch-fed matmuls through the DMA ramp so
            # the HAM clock gate reaches 8/8 before the real stream ----
            nc.vector.memset(scr[:], 0.0)

            def junk(n):
                for _ in range(n):
                    t = ps_st.tile([P, 512], F32, name="st", tag="st")
                    nc.tensor.matmul(t[:], scr[:, 0:P], scr[:],
                                     start=True, stop=True)

            junk(10)

            # ---- input DMAs (identity copies, issued sync/gpsimd) ----
            # per k-block first: x then the qk-e-tile-0 slice, so the first
            # QKV accumulation ramps with the DMA stream; v weights next
            # (attn@V needs them by ~step LAG), then the rest.
            engs = [nc.sync, nc.gpsimd]
            q = 0

            def dma(dst, src):
                nonlocal q
                engs[q % 2].dma_start(dst, src)
                q += 1

            for kb in range(NB):
                dma(xtw[:, kb * L:(kb + 1) * L],
                    xt_d[:, kb * L:(kb + 1) * L])
                dma(wqs(kb, 0, 2 * P),
                    wqkv_d[:, kb * WQW:kb * WQW + 2 * P])
            for kb in range(NB):
                dma(wqs(kb, VOFF, DPC),
                    wqkv_d[:, kb * WQW + VOFF:kb * WQW + VOFF + DPC])
            for kb in range(NB):
                dma(wqs(kb, 2 * P, VOFF - 2 * P),
                    wqkv_d[:, kb * WQW + 2 * P:kb * WQW + VOFF])
            dma(wpw[:], wproj_d[:])

            # ---- generators for fill work ----
            def emit_qk_et(et):
                for which, dst in ((0, qt), (1, kt)):
                    for ch in range(2):
                        acc = ps_sm.tile([P, 512], F32, name="psqk", tag="ps")

                        def mk(acc=acc, which=which, et=et, ch=ch):
                            for kb in range(NB):
                                yield lambda kb=kb, acc=acc, which=which, \
                                    et=et, ch=ch: nc.tensor.matmul(
                                    acc[:],
                                    wqs(kb, et * 2 * P + which * P, P),
                                    xts(kb, ch * 512, 512),
                                    start=(kb == 0), stop=(kb == NB - 1),
                                )
                        yield from mk()
                        if et == 0:
                            yield lambda dst=dst, et=et, ch=ch, acc=acc: \
                                nc.scalar.copy(
                                    dst[et][:, ch * 512:(ch + 1) * 512],
                                    acc[:])
                        else:
                            yield lambda dst=dst, et=et, ch=ch, acc=acc: \
                                nc.vector.tensor_copy(
                                    dst[et][:, ch * 512:(ch + 1) * 512],
                                    acc[:])

            def emit_v(lb):
                yield lambda lb=lb: nc.vector.memset(
                    vsb[lb][:, :, HD:VW], 1.0)
                acc = ps_sm.tile([P, DPC], F32, name="psv", tag="ps")

                def mk(acc=acc, lb=lb):
                    for kb in range(NB):
                        yield lambda kb=kb, acc=acc, lb=lb: nc.tensor.matmul(
                            acc[:],
                            xts(kb, lb * P, P),
                            wqs(kb, VOFF, DPC),
                            start=(kb == 0), stop=(kb == NB - 1),
                        )
                yield from mk()
                yield lambda acc=acc, lb=lb: nc.vector.tensor_copy(
                    vsb[lb][:, :, 0:HD],
                    acc[:].rearrange("p (h d) -> p h d", h=HPC),
                )

            # Output blocks are packed ch-outer (block = ch*NB + et, host
            # re-orders), so consecutive et tiles of a quarter are adjacent
            # in DRAM: two casts stage into one [128, 1024] buffer and ship
            # as a single store - half the DMA issues.
            def emit_proj(dbs, ydst, sync_store):
                # ch=0 tiles first: the ch=1 normalize multiplies of the
                # gating pair have just been emitted when this generator
                # opens, so ch=1 readers must trail them by several tiles.
                stage = [None]
                for u, (ch, et) in enumerate(
                        (ch, et) for ch in range(2) for et in range(NB)):
                    acc = ps_sm.tile([P, 512], F32, name="psy", tag="ps")
                    for db in dbs:
                        yield lambda acc=acc, db=db, et=et, ch=ch, dbs=dbs: \
                            nc.tensor.matmul(
                                acc[:],
                                wps(db, et * P, P),
                                osb[db][:, ch * 512:(ch + 1) * 512],
                                start=(db == dbs[0]), stop=(db == dbs[-1]),
                            )

                    def fin(acc=acc, et=et, ch=ch, ydst=ydst, u=u,
                            sync_store=sync_store, stage=stage):
                        if et % 2 == 0:
                            stage[0] = ysp.tile([P, 1024], BF, name="ystage",
                                                tag="ystage")
                        dst = stage[0][:, (et % 2) * 512:(et % 2 + 1) * 512]
                        # the late quarter's ch=1 tiles drain in the flush,
                        # when ScalarE is done with exps - offload its casts
                        # there to unclog the DVE cast pipe in that window
                        if sync_store and ch == 1:
                            nc.scalar.copy(dst, acc[:])
                        else:
                            nc.vector.tensor_copy(dst, acc[:])
                        if et % 2 == 1:
                            eng = (nc.sync if (sync_store or (u // 2) % 2)
                                   else nc.gpsimd)
                            eng.dma_start(
                                ydst[:, (ch * NB + et - 1) * 512:
                                     (ch * NB + et + 1) * 512],
                                stage[0][:])
                    yield fin

            # pair-3 projection accumulators round-robin across all three
            # PSUM pools (score and oacc slots are dead by the tail) so the
            # matmuls never wait on an eviction cast.
            tail_pools = [ps_sm, ps_st, ps_o]
            tail_tags = ["ps", "st", "ot"]

            def emit_proj3g(ch):
                # ch=0 runs while scores still cycle the st pool and pair-3
                # ch=1 still accumulates in the ot pool - keep its
                # accumulators in the sm pool. ch=1 runs in the flush when
                # both other pools are dead.
                stage = [None]
                for et in range(NB):
                    pi = 0 if ch == 0 else 1 + et % 2
                    acc = tail_pools[pi].tile([P, 512], F32, name="psy3",
                                              tag=tail_tags[pi])
                    yield lambda acc=acc, et=et, ch=ch: nc.tensor.matmul(
                        acc[:], wps(3, et * P, P),
                        osb[3][:, ch * 512:(ch + 1) * 512],
                        start=True, stop=True,
                    )

                    def fin(acc=acc, et=et, ch=ch, stage=stage):
                        if et % 2 == 0:
                            stage[0] = ysp.tile([P, 1024], BF, name="ystage",
                                                tag="ystage")
                        if ch == 1 and et % 2:
                            nc.scalar.copy(
                                stage[0][:, 512:1024], acc[:])
                        else:
                            nc.vector.tensor_copy(
                                stage[0][:, (et % 2) * 512:
                                         (et % 2 + 1) * 512],
                                acc[:])
                        if et % 2 == 1:
                            nc.sync.dma_start(
                                yq_d[1][:, (ch * NB + et - 1) * 512:
                                        (ch * NB + et + 1) * 512],
                                stage[0][:])
                    yield fin

            # ---- EDF filler scheduling ----
            # fillers: [deadline_substep, ops_left, gate, gen]; gate 0-3
            # compares against pairs evicted, gate 4/5 against pair-3
            # normalize halves done.
            NHS = 64
            evicted = [0]
            p3 = [0]
            fillers = []
            for jb in range(NB):
                fillers.append([max(1, LAG + jb - 1), 10, 0, emit_v(jb)])
            for g in (1, 2, 3):
                fillers.append([16 * g - 2, 36, 0, emit_qk_et(g)])
            # proj quarters stream as their pair groups evict; quarter [2]'s
            # deadline sits past the last step so leftovers keep the PE warm
            # through the tail eviction.
            fillers.append([54, 48, 2, emit_proj([0, 1], yq_d[0], False)])
            fillers.append([NHS + 10, 32, 3, emit_proj([2], yq_d[2], True)])
            fillers.append([NHS + 4, 16, 4, emit_proj3g(0)])
            fillers.append([NHS + 12, 16, 5, emit_proj3g(1)])

            def gate_ok(f):
                g = f[2]
                return evicted[0] >= g if g <= 3 else p3[0] >= g - 3

            def drain_one():
                for f in fillers:
                    if f[1] == 0 or not gate_ok(f):
                        continue
                    fn = next(f[3], None)
                    if fn is None:
                        f[1] = 0
                        continue
                    fn()
                    f[1] -= 1
                    return True
                return False

            def drain_for(t):
                for f in fillers:
                    while f[0] <= t and f[1] > 0 and gate_ok(f):
                        fn = next(f[3], None)
                        if fn is None:
                            f[1] = 0
                            break
                        fn()
                        f[1] -= 1
                due = sorted((f for f in fillers
                              if f[1] > 0 and gate_ok(f)),
                             key=lambda f: f[0])
                n = 2
                cum = 0
                for f in due:
                    cum += f[1]
                    if f[0] > t:
                        n = max(n, -(-cum // (f[0] - t)))
                for _ in range(min(n, 9)):
                    if not drain_one():
                        break

            # ---- phase 1: q/k e-tile 0 ----
            # kb-outer with all four accumulators live (2 sm + 2 st slots):
            # each k-block's four matmuls consume its x/weight DMAs right as
            # they land instead of re-traversing all blocks per output, and
            # each weight load serves both i-chunks back-to-back.
            p1acc = {}
            for which in range(2):
                for ch in range(2):
                    pool, tag = ((ps_sm, "ps") if which == 0
                                 else (ps_st, "st"))
                    p1acc[(which, ch)] = pool.tile([P, 512], F32,
                                                   name="psqk0", tag=tag)
            for kb in range(NB):
                for which in range(2):
                    for ch in range(2):
                        nc.tensor.matmul(
                            p1acc[(which, ch)][:],
                            wqs(kb, which * P, P),
                            xts(kb, ch * 512, 512),
                            start=(kb == 0), stop=(kb == NB - 1),
                        )
            for which, dst in ((0, qt), (1, kt)):
                for ch in range(2):
                    nc.scalar.copy(dst[0][:, ch * 512:(ch + 1) * 512],
                                   p1acc[(which, ch)][:])

            # ---- phase 2: paired attention with lagged attn@V ----
            # ch-major sub-steps: each (g, ch, jb) sub-step computes one
            # [128j x 512i] score block for both heads. The per-(head, ch)
            # attn@V accumulator is a single-bank [65, 512] PSUM tile, which
            # frees banks for a 3-deep score-tile pipeline (the next score
            # matmul never waits on this sub-step's exp).
            substeps = [(g, ch, jb)
                        for g in range(4) for ch in range(2)
                        for jb in range(NB)]
            emis = {}
            for idx, (g, ch, jb) in enumerate(substeps):
                at = idx + LAG + (1 if jb == 0 else 0)
                emis.setdefault(at, []).append((g, ch, jb))
            ptt = {}
            oacc = {}

            def emit_attnv(g2, ch2, jb2):
                for h, hi in ((2 * g2, 0), (2 * g2 + 1, 1)):
                    if jb2 == 0:
                        oacc[(h, ch2)] = ps_o.tile([VW, 512], F32,
                                                   name="ot", tag="ot")
                    pt = ptt.pop((g2, ch2, jb2, hi))
                    nc.tensor.matmul(
                        oacc[(h, ch2)][:],
                        vsb[jb2][:, h, :],
                        pt[:],
                        start=(jb2 == 0), stop=(jb2 == NB - 1),
                    )

            def emit_evict_chunk(g2, ch2):
                # Normalize the ch-half of both heads of pair g2 straight out
                # of their PSUM accumulators (no staging copy).
                sl = slice(ch2 * 512, (ch2 + 1) * 512)
                for h in (2 * g2, 2 * g2 + 1):
                    base = (h % 2) * HD
                    acc = oacc[(h, ch2)]
                    sums = work.tile([1, 512], F32, name="sums", tag="sums")
                    nc.vector.tensor_copy(sums[:], acc[HD:HD + 1, :])
                    inv = work.tile([1, 512], F32, name="inv", tag="inv")
                    nc.vector.reciprocal_approx_fast(inv[:], sums[:])
                    invbc = work.tile([HD, 512], F32, name="invbc",
                                      tag="invbc")
                    nc.gpsimd.partition_broadcast(invbc[:], inv[:])
                    nc.vector.tensor_mul(
                        osb[g2][base:base + HD, sl],
                        acc[0:HD, :],
                        invbc[:],
                    )
                if ch2 == 1:
                    evicted[0] = g2 + 1
                if g2 == 3:
                    p3[0] = ch2 + 1

            for si, (g, ch, jb) in enumerate(substeps):
                tq, tk = qt[g], kt[g]
                for g2, ch2, jb2 in emis.get(si, ()):
                    emit_attnv(g2, ch2, jb2)
                    if jb2 == NB - 1:
                        emit_evict_chunk(g2, ch2)
                for hi, rows in ((0, slice(0, HD)), (1, slice(HD, P))):
                    stx = ps_st.tile([P, 512], F32, name="st", tag="st")
                    nc.tensor.matmul(
                        stx[:],
                        tk[rows, jb * P:(jb + 1) * P],
                        tq[rows, ch * 512:(ch + 1) * 512],
                        start=True, stop=True,
                    )
                    pt = ptp.tile([P, 512], BF, name="pt", tag="pt")
                    nc.scalar.activation(pt[:], stx[:], Exp)
                    ptt[(g, ch, jb, hi)] = pt
                drain_for(si)

            # ---- phase 3: flush lag, final evict + tail projection ----
            for si in range(len(substeps), len(substeps) + LAG + 2):
                for g2, ch2, jb2 in emis.get(si, ()):
                    emit_attnv(g2, ch2, jb2)
                    if jb2 == NB - 1:
                        emit_evict_chunk(g2, ch2)
                    for _ in range(3):
                        drain_one()
            while any(f[1] for f in fillers if gate_ok(f)):
                if not drain_one():
                    break

    nc.compile()
    return nc


def _get_nc():
    if "nc" not in _CACHE:
        _CACHE["nc"] = _build_nc()
    return _CACHE["nc"]


def kernel(x, w_qkv, w_proj, b_proj, ps_w1, ps_b1, ln_g, ln_b, ps_w2, ps_b2,
           ps_w3, ps_b3, pattern_bias, sparse_w, sparse_b):
    import concourse.mybir as mybir
    from concourse.bass_utils import run_bass_kernel_spmd

    pw = _pattern_weights(x, ps_w1, ps_b1, ln_g, ln_b, ps_w2, ps_b2, ps_w3,
                          ps_b3, pattern_bias)
    if pw[:, 1].min() <= THRESHOLD + 1e-4:
        # mask not provably dense -> exact (slow) fallback
        return _numpy_reference(x, w_qkv, w_proj, b_proj, ps_w1, ps_b1, ln_g,
                                ln_b, ps_w2, ps_b2, ps_w3, ps_b3, pattern_bias,
                                sparse_w, sparse_b)

    bf16 = mybir.dt.np(mybir.dt.bfloat16)
    x = np.asarray(x, np.float32)
    w_qkv = np.asarray(w_qkv, np.float32)
    w_proj = np.asarray(w_proj, np.float32)

    in_maps = []
    for c in range(N_CORES):
        b = c // 2
        h0 = (c % 2) * HPC
        rq = slice(h0 * HD, (h0 + HPC) * HD)
        wqc = w_qkv[0 * D:1 * D][rq] * SCALE           # [512, 1024]
        wkc = w_qkv[1 * D:2 * D][rq]
        wvc = w_qkv[2 * D:3 * D][rq]
        # packed row order: [q_et0, k_et0, q_et1, k_et1, ..., v]
        secs = []
        for et in range(4):
            secs.append(wqc[et * P:(et + 1) * P])
            secs.append(wkc[et * P:(et + 1) * P])
        secs.append(wvc)
        wqkvT = np.concatenate(secs, 0).T.astype(bf16)          # [1024, 1536]
        wprojT = w_proj.T[rq, :].astype(bf16)                   # [512, 1024]
        xt = x[b].T.astype(bf16)                                # [1024, 1024]
        in_maps.append({"xtP": _pmajor(xt), "wqkvP": _pmajor(wqkvT),
                        "wprojP": _pmajor(wprojT)})

    res = run_bass_kernel_spmd(_get_nc(), in_maps, list(range(N_CORES)),
                               trace=bool(_CACHE.get("trace", False)))
    _CACHE["last_exec_time_ns"] = res.exec_time_ns
    _CACHE["last_res"] = res

    out = np.empty((B, L, D), np.float32)
    bp = np.asarray(b_proj, np.float32)
    for b in range(B):
        yt = np.zeros((P, NB * L), np.float32)
        for c in (2 * b, 2 * b + 1):
            r = res.results[c]
            for i in range(3):
                yt += r[f"yQ{i}"].astype(np.float32)
        # device packs output blocks ch-outer: block = ch*NB + et
        yt = yt.reshape(P, 2, NB, 512).transpose(0, 2, 1, 3).reshape(
            P, NB * L)
        out[b] = _unpmajor(yt, D).T + bp[None, :]
    return out
